# revision 29
# baseline (speedup 1.0000x reference)
"""Entropy-gated multi-head attention on 8 Trainium2 NeuronCores.

Sharding: core c = b*4 + g handles batch b (of 2) and head-group g (4 of the
16 heads).  Tokens with gate==0 pass x through untouched and contribute
exactly zero k/v (zero biases), so the device only processes the compacted
active tokens (~half), with the softmax denominator corrected by the count of
inactive tokens: each inactive key contributes exp(0)=1 to the softmax sum
(scores vs. zeroed k are exactly 0) and nothing to the numerator (v=0).

Device math per core (no max-subtraction; scores are O(5) so exp is safe):
  QT = Wq_g^T x^T, KT = Wk_g^T x^T           [256, SA]
  V  = x Wv_g                                 [SA, 256]
  per head h: PT = exp((KT_h^T QT_h)/8)/16    [SA_k, SA_q]  (fp8, DoubleRow)
              OT' = [V_h | 1]^T PT            [65, SA_q] (row 64 = colsum)
              r = 1/(Z + (S - SA)/16)         broadcast to [64, SA_q] via DMA
              osb_h = OT * r                  (scaled attention out, lhsT form)
  Y(q, :) = sum_h osb_h^T Wo_h                (K=128 pair-packed psum accum)
Host sums the 4 per-group partial Y per batch, adds bo, scatters into x.

fp8 path: scores contract DH=64 as [32 partitions x 2 k-subtiles] DoubleRow
(W columns host-permuted so QT/KT psum halves land directly in the
[128, 2, SA] layout); OT contracts key-tile pairs as [128, 2, 65] DoubleRow
with PT written by ACT as exp(s/8)/16 in fp8e4 (1/16 keeps exp under the
+-240 fp8e4 clip; the softmax correction uses CADD/16 to compensate).
"""

import math
from contextlib import ExitStack

import numpy as np
import ml_dtypes

import concourse.bass as bass
import concourse.mybir as mybir
from concourse import bacc
import concourse.tile as tile
from concourse.bass_utils import run_bass_kernel_spmd

B, S, D = 2, 2048, 1024
H, DH = 16, 64
NCORES = 8
GROUPS = NCORES // B          # head-groups per batch = 4
HC = H // GROUPS              # heads per core = 4
DC = HC * DH                  # head-group width = 256

# matmul operand dtype for projections/Y: "bf16" | "f32" | "f32r"
MM_DTYPE = "bf16"
SCORE_FP8 = False             # 32-row DR scores stream slower than bf16: keep bf16
OT_FP8 = True                 # fp8e4 pt/v_aug, DoubleRow over key-tile pairs
PROJ_FP8 = True               # fp8e4 x/Wq/Wk/Wv, DoubleRow over k-tile pairs
Y_FP8 = True                  # fp8e4 osb/Wo, one DoubleRow matmul per (jt,dch)
# W host-scale 16 keeps fp8 weights in the normal range; compensated by
# exp-scale/256 (q,k both x16) and the osb output scale 1/16 (v x16)
WSCALE = 16.0

_DT = {
    "bf16": mybir.dt.bfloat16,
    "f32": mybir.dt.float32,
    "f32r": mybir.dt.float32,
}
_NPDT = {
    "bf16": ml_dtypes.bfloat16,
    "f32": np.float32,
    "f32r": np.float32,
}

f32 = mybir.dt.float32
fp8 = mybir.dt.float8e4
LN16 = float(np.log(16.0))
DR = mybir.MatmulPerfMode.DoubleRow


def _chunks(total, step):
    out = []
    o = 0
    while o < total:
        out.append((o, min(step, total - o)))
        o += step
    return out


def _build(SA: int, dtype_tag: str) -> bass.Bass:
    DT = _DT[dtype_tag]
    SDT = fp8 if SCORE_FP8 else DT   # qt/kt storage
    PDT = fp8 if OT_FP8 else DT      # pt/v_aug storage
    XDT = fp8 if PROJ_FP8 else DT    # x / Wq / Wk / Wv storage
    PT_SCALE = 16.0 if OT_FP8 else 1.0
    if PROJ_FP8:
        assert OT_FP8, "PROJ_FP8 requires the fp8 OT path (scale bookkeeping)"
    # with PROJ_FP8, q/k carry a WSCALE factor each -> scores are WSCALE^2 up
    ESCALE = 0.125 / (WSCALE * WSCALE if PROJ_FP8 else 1.0)

    def mm(ap):
        if dtype_tag == "f32r":
            return ap.bitcast(mybir.dt.float32r)
        return ap

    nkt = D // 128            # 8 contraction tiles for projections
    nst = SA // 128           # token tiles
    qch = _chunks(SA, 512)    # q chunks
    dch = _chunks(D, 512)     # output-dim chunks
    CADD = float(S - SA) / PT_SCALE

    nc = bacc.Bacc()
    xT_d = nc.dram_tensor("xT", [D, SA], XDT, kind="ExternalInput")
    wq_d = nc.dram_tensor("wq", [D, DC], XDT, kind="ExternalInput")
    wk_d = nc.dram_tensor("wk", [D, DC], XDT, kind="ExternalInput")
    wv_d = nc.dram_tensor("wv", [D, DC], XDT, kind="ExternalInput")
    WODT = fp8 if Y_FP8 else DT
    wo_d = nc.dram_tensor("wo", [DC, D], WODT, kind="ExternalInput")
    y_d = nc.dram_tensor("y", [SA, D], f32, kind="ExternalOutput")

    with tile.TileContext(nc) as tc, ExitStack() as ctx:
        singles = ctx.enter_context(tc.tile_pool(name="singles", bufs=1))
        pt_pool = ctx.enter_context(tc.tile_pool(name="pt", bufs=6))
        otsb_pool = ctx.enter_context(tc.tile_pool(name="otsb", bufs=2))
        zr_pool = ctx.enter_context(tc.tile_pool(name="zr", bufs=2))
        zq_pool = ctx.enter_context(tc.tile_pool(name="zq", bufs=2))
        rbc_pool = ctx.enter_context(tc.tile_pool(name="rbc", bufs=2))
        yout_pool = ctx.enter_context(tc.tile_pool(name="yout", bufs=3))
        zscr_pool = ctx.enter_context(tc.tile_pool(name="zscr", bufs=8,
                                                   space="DRAM"))
        # PSUM: st pairs 2x2 banks + aux (proj/V/Y) 2x1 + ot 2x1 = 8 banks
        mm_ps = ctx.enter_context(tc.tile_pool(name="mmps", bufs=2, space="PSUM"))
        aux_ps = ctx.enter_context(tc.tile_pool(name="auxps", bufs=2, space="PSUM"))
        ot_ps_pool = ctx.enter_context(tc.tile_pool(name="otps", bufs=2, space="PSUM"))

        # ---- persistent SBUF; one batched DMA per tensor/chunk keeps the
        # sync-queue issue time (~0.6us per dma_start) off the startup path
        wq_sb = singles.tile([128, nkt, DC], XDT)
        wk_sb = singles.tile([128, nkt, DC], XDT)
        wv_sb = singles.tile([128, nkt, DC], XDT)
        xt = singles.tile([128, nkt, SA], XDT)
        (c0, c0n) = qch[0]
        nc.sync.dma_start(wq_sb[:, :, :],
                          wq_d[:, :].rearrange("(t p) c -> p t c", p=128))
        nc.sync.dma_start(xt[:, :, c0:c0 + c0n],
                          xT_d[:, c0:c0 + c0n].rearrange(
                              "(t p) q -> p t q", p=128))
        nc.sync.dma_start(wk_sb[:, :, :],
                          wk_d[:, :].rearrange("(t p) c -> p t c", p=128))
        for (q0, qn) in qch[1:]:
            nc.sync.dma_start(xt[:, :, q0:q0 + qn],
                              xT_d[:, q0:q0 + qn].rearrange(
                                  "(t p) q -> p t q", p=128))
        nc.sync.dma_start(wv_sb[:, :, :],
                          wv_d[:, :].rearrange("(t p) c -> p t c", p=128))
        if Y_FP8:
            # [128, 2, D]: pair p rows as DR subtile p
            wo2 = singles.tile([128, 2, D], WODT, tag="wo2", name="wo2")
            nc.sync.dma_start(wo2[:, :, :],
                              wo_d[:, :].rearrange("(p k) d -> k p d", p=2))
            wo_sb = []
        else:
            wo_sb = []
            for p in range(HC // 2):
                w = singles.tile([128, D], DT, tag=f"wo{p}", name=f"wo{p}")
                nc.sync.dma_start(w, wo_d[p * 128:(p + 1) * 128, :])
                wo_sb.append(w)

        # ---- projections ----
        if SCORE_FP8:
            # [128, 2, SA]: head h on partitions 32h..32h+32, dim d of head h
            # at (p = 32h + d%32, j = d//32); W cols are host-permuted so the
            # m-th psum half is exactly the j=m slice.
            qt = singles.tile([128, 2, SA], SDT, tag="qt", name="qt")
            kt = singles.tile([128, 2, SA], SDT, tag="kt", name="kt")
        else:
            qt = [singles.tile([128, SA], DT, tag=f"qt{m}", name=f"qt{m}")
                  for m in range(2)]
            kt = [singles.tile([128, SA], DT, tag=f"kt{m}", name=f"kt{m}")
                  for m in range(2)]
        # 68 cols (V | ones | zero-pad): the dual-fp8 LDWEIGHTS subtile
        # stride (HC*cols) must be a multiple of 16, so cols % 4 == 0
        v_aug = singles.tile([128, nst, HC, 68], PDT)

        def proj_qk(m, dst, w_sb, q0, qn):
            ps = aux_ps.tile([128, 512], f32, tag="aux", name="ps")
            if PROJ_FP8:
                for t2 in range(nkt // 2):
                    nc.tensor.matmul(
                        ps[:, :qn],
                        w_sb[:, 2 * t2:2 * t2 + 2, m * 128:(m + 1) * 128],
                        xt[:, 2 * t2:2 * t2 + 2, q0:q0 + qn],
                        start=(t2 == 0), stop=(t2 == nkt // 2 - 1),
                        perf_mode=DR)
            else:
                for t in range(nkt):
                    nc.tensor.matmul(
                        ps[:, :qn],
                        mm(w_sb[:, t, m * 128:(m + 1) * 128]),
                        mm(xt[:, t, q0:q0 + qn]),
                        start=(t == 0), stop=(t == nkt - 1))
            if SCORE_FP8:
                # m-half m = heads (2m, 2m+1): psum [0:64] is their j=0
                # subtile, [64:128] their j=1 (cross-partition-base copy)
                nc.vector.tensor_copy(dst[64 * m:64 * m + 64, 0, q0:q0 + qn],
                                      ps[0:64, :qn])
                nc.vector.tensor_copy(dst[64 * m:64 * m + 64, 1, q0:q0 + qn],
                                      ps[64:128, :qn])
            else:
                nc.vector.tensor_copy(dst[m][:, q0:q0 + qn], ps[:, :qn])

        v_done = set()

        def proj_v(s):
            v_done.add(s)
            ps = aux_ps.tile([128, 512], f32, tag="aux", name="ps")
            if PROJ_FP8:
                for t2 in range(nkt // 2):
                    nc.tensor.matmul(
                        ps[:, :DC],
                        xt[:, 2 * t2:2 * t2 + 2, s * 128:(s + 1) * 128],
                        wv_sb[:, 2 * t2:2 * t2 + 2, :],
                        start=(t2 == 0), stop=(t2 == nkt // 2 - 1),
                        perf_mode=DR)
            else:
                for t in range(nkt):
                    nc.tensor.matmul(
                        ps[:, :DC],
                        mm(xt[:, t, s * 128:(s + 1) * 128]),
                        mm(wv_sb[:, t, :]),
                        start=(t == 0), stop=(t == nkt - 1))
            for h in range(HC):
                nc.vector.tensor_copy(v_aug[:, s, h, 0:64],
                                      ps[:, h * 64:(h + 1) * 64])

        # m=0 projections and the first V tiles run up-front (PE-dense, warms
        # HAM); remaining independent PE work (V tail, m=1 projections, Y of
        # finished chunks) is queued and drained between attention groups so
        # the PE never starves while ACT runs the exps.
        aux_jobs = []

        def drain_aux(k):
            for _ in range(min(k, len(aux_jobs))):
                aux_jobs.pop(0)()

        nc.vector.memset(v_aug[:, :, :, 64:65], 1.0)
        nc.vector.memset(v_aug[:, :, :, 65:68], 0.0)
        ebias = None
        if OT_FP8:
            # exp bias -ln16: pt = exp(s/8)/16 keeps exp under the fp8e4 clip
            ebias = singles.tile([128, 1], f32, tag="ebias", name="ebias")
            nc.vector.memset(ebias, -LN16)
        for (q0, qn) in qch:
            proj_qk(0, qt, wq_sb, q0, qn)
            proj_qk(0, kt, wk_sb, q0, qn)
        proj_v(0)
        proj_v(1)
        for s in range(2, nst):
            aux_jobs.append(lambda s=s: proj_v(s))
        for (q0, qn) in qch:
            aux_jobs.append(lambda a=q0, b=qn: proj_qk(1, qt, wq_sb, a, b))
            aux_jobs.append(lambda a=q0, b=qn: proj_qk(1, kt, wk_sb, a, b))

        # ---- attention + output projection, per q chunk ----
        for ci, (q0, qn) in enumerate(qch):
            ot_sb = [None] * HC
            osball = (otsb_pool.tile([128, 2, 512], fp8, tag="osball",
                                     name="osball") if Y_FP8 else None)
            for p in range(HC // 2):
                m = p
                if p == 1 and ci == 0:
                    # pair 1 needs the m=1 projections: pull them forward
                    drain_aux(len(aux_jobs))
                ot_ps = {}
                for h in (2 * p, 2 * p + 1):
                    ot_ps[h] = ot_ps_pool.tile([68, 512], f32, tag="ot",
                                               name="ot_ps")
                for si in range(0, nst, 2):
                    # NOTE: emission order is semantic order in Tile — a
                    # consumer emitted before its producer reads stale data.
                    # During chunk0/pair0 the V-projection jobs at the queue
                    # head MUST outpace the OT consumers: 2 jobs per si-group
                    # keeps proj_v(s) strictly ahead of OT reads of v_aug[s].
                    drain_aux(2 if (ci == 0 and p == 0) else 1)
                    npair = min(2, nst - si)
                    assert all(s in v_done for s in range(si, si + npair)), \
                        f"proj_v not emitted before OT consumer: {si}"

                    st, pt = {}, {}
                    for h in (2 * p, 2 * p + 1):
                        st[h] = mm_ps.tile([128, 2, 512], f32, tag="mm",
                                           name="st_ps")
                    for j in range(npair):
                        s = si + j
                        for h in (2 * p, 2 * p + 1):
                            if SCORE_FP8:
                                nc.tensor.matmul(
                                    st[h][:, j, :qn],
                                    kt[32 * h:32 * h + 32, :,
                                       s * 128:(s + 1) * 128],
                                    qt[32 * h:32 * h + 32, :, q0:q0 + qn],
                                    start=True, stop=True,
                                    perf_mode=DR,
                                    tile_position=(32 * h, 0))
                            else:
                                # adjacent (even,odd) matmuls with explicit
                                # row-group tile_position pack the PE array
                                r0 = (h % 2) * 64
                                nc.tensor.matmul(
                                    st[h][:, j, :qn],
                                    mm(kt[m][r0:r0 + 64,
                                             s * 128:(s + 1) * 128]),
                                    mm(qt[m][r0:r0 + 64, q0:q0 + qn]),
                                    start=True, stop=True,
                                    tile_position=(r0, 0))
                    for h in (2 * p, 2 * p + 1):
                        pt[h] = pt_pool.tile([128, 2, 512], PDT, tag="pt",
                                             name="pt")
                        nc.scalar.activation(
                            pt[h][:, :npair, :qn], st[h][:, :npair, :qn],
                            mybir.ActivationFunctionType.Exp,
                            scale=ESCALE,
                            bias=(ebias[:, :] if OT_FP8 else 0.0))
                    for h in (2 * p, 2 * p + 1):
                        if OT_FP8 and npair == 2:
                            nc.tensor.matmul(
                                ot_ps[h][:, :qn],
                                v_aug[:, si:si + 2, h, :],
                                pt[h][:, 0:2, :qn],
                                start=(si == 0), stop=(si + 1 == nst - 1),
                                perf_mode=DR)
                        else:
                            for j in range(npair):
                                s = si + j
                                nc.tensor.matmul(
                                    ot_ps[h][:, :qn],
                                    mm(v_aug[:, s, h, 0:68]) if not OT_FP8
                                    else v_aug[:, s, h, 0:68],
                                    mm(pt[h][:, j, :qn]) if not OT_FP8
                                    else pt[h][:, j, :qn],
                                    start=(s == 0), stop=(s == nst - 1))
                if Y_FP8:
                    osbp = osball[:, p, :]
                else:
                    osbp = otsb_pool.tile([128, 512], DT, tag=f"osbp{p}",
                                          name=f"osbp{p}")
                ot_sb[p] = osbp
                for h in (2 * p, 2 * p + 1):
                    # evacuate psum immediately (zt row + unscaled OT copy,
                    # both cheap DVE) so the bank frees for the next chunk's
                    # OT; the r chain (gpsimd-queue DMA hops) then runs off
                    # the critical path; osb is scaled before the Y matmuls.
                    r0 = 0 if h % 2 == 0 else 64
                    zt = zr_pool.tile([65, 512], f32, tag="zt", name="zt")
                    nc.vector.tensor_scalar(
                        out=zt[64:65, :qn], in0=ot_ps[h][64:65, :qn],
                        scalar1=CADD, scalar2=None, op0=mybir.AluOpType.add)
                    otu = otsb_pool.tile([128, 512], DT,
                                         tag=f"otu{h - 2 * p}", name="otu")
                    nc.vector.tensor_copy(otu[r0:r0 + 64, :qn],
                                          ot_ps[h][0:64, :qn])
                    # z [1,qn] -> [128,4] so the reciprocal runs wide on DVE;
                    # alternate issue queue by head parity so the two chains
                    # of a pair don't serialize on one sequencer at endgame
                    dq = nc.gpsimd if h % 2 == 0 else nc.sync
                    zq = zq_pool.tile([128, 4], f32, tag="zq", name="zq")
                    qp = (qn + 3) // 4
                    dq.dma_start(zq[:qp, :], zt[64:65, :qn])
                    nc.vector.reciprocal(zq[:qp, :], zq[:qp, :])
                    zd2 = zscr_pool.tile([1, 512], f32, tag="zd2", name="zd2")
                    dq.dma_start(zd2[0:1, :qn], zq[:qp, :])
                    rb = rbc_pool.tile([128, 512], f32, tag=f"rbc{h}",
                                       name=f"rbc{h}")
                    dq.dma_start(rb[r0:r0 + 64, :qn],
                                 zd2[0:1, :qn].to_broadcast((64, qn)))
                    nc.vector.scalar_tensor_tensor(
                        out=osbp[r0:r0 + 64, :qn],
                        in0=otu[r0:r0 + 64, :qn],
                        scalar=(1.0 / WSCALE if PROJ_FP8 else 1.0),
                        in1=rb[r0:r0 + 64, :qn],
                        op0=mybir.AluOpType.mult,
                        op1=mybir.AluOpType.mult)

            # output projection: osbp packs the head pair on 128 partitions,
            # so each pair is a single K=128 accumulating matmul per output
            # chunk.  p-outer/dch-inner keeps the stationary operand
            # back-to-back for weight-reuse.  Queued so the Y matmuls fill
            # PE bubbles of the next chunk's (ACT-bound) attention.
            def y_job(q0, qn, jt, osb_pair, osball=None):
                qtn = min(128, qn - jt * 128)
                yps = [aux_ps.tile([128, 512], f32, tag="aux", name=f"y{di}")
                       for di in range(len(dch))]
                for di, (d0, dn) in enumerate(dch):
                    if Y_FP8:
                        nc.tensor.matmul(
                            yps[di][:qtn, :dn],
                            osball[:, :, jt * 128:jt * 128 + qtn],
                            wo2[:, :, d0:d0 + dn],
                            start=True, stop=True, perf_mode=DR)
                    else:
                        for p in range(HC // 2):
                            nc.tensor.matmul(
                                yps[di][:qtn, :dn],
                                mm(osb_pair[p][:, jt * 128:jt * 128 + qtn]),
                                mm(wo_sb[p][:, d0:d0 + dn]),
                                start=(p == 0), stop=(p == HC // 2 - 1))
                for di, (d0, dn) in enumerate(dch):
                    yo = yout_pool.tile([128, 512], f32, tag="yo", name="yo")
                    if Y_FP8:
                        nc.vector.tensor_scalar(
                            out=yo[:qtn, :dn], in0=yps[di][:qtn, :dn],
                            scalar1=1.0 / WSCALE, scalar2=None,
                            op0=mybir.AluOpType.mult)
                    else:
                        nc.vector.tensor_copy(yo[:qtn, :dn], yps[di][:qtn, :dn])
                    # scalar queue: the sync queue is congested with
                    # end-of-kernel semaphores right when the last y lands
                    nc.scalar.dma_start(
                        y_d[q0 + jt * 128: q0 + jt * 128 + qtn, d0:d0 + dn],
                        yo[:qtn, :dn])

            for jt in range((qn + 127) // 128):
                aux_jobs.append(
                    lambda a=q0, b=qn, j=jt, o=tuple(ot_sb[:HC // 2]),
                    ob=osball: y_job(a, b, j, o, ob))
        drain_aux(len(aux_jobs))
    nc.compile()
    return nc


_nc_cache: dict = {}


def _get_nc(SA: int):
    key = (SA, MM_DTYPE, SCORE_FP8, OT_FP8, PROJ_FP8, Y_FP8)
    if key not in _nc_cache:
        _nc_cache[key] = _build(SA, MM_DTYPE)
    return _nc_cache[key]


def _score_perm():
    """W column permutation (within each head-group's 256 cols): m-half m
    covers heads (2m, 2m+1); psum partitions [0:64] are their j=0 subtile
    (dims 0..31), [64:128] the j=1 subtile (dims 32..63), so head h sits at
    qt[32h:32h+32, j, :] with dim d = j*32 + p%32."""
    perm = np.empty(2 * 128, np.int64)
    for m in range(2):
        for p in range(128):
            h = 2 * m + (p % 64) // 32
            d = (p // 64) * 32 + (p % 32)
            perm[m * 128 + p] = h * 64 + d
    return perm


def _reference_fallback(x, gate, Wq, bq, Wk, bk, Wv, bv, Wo, bo):
    g = gate.astype(x.dtype)[..., None]
    q = (x @ Wq + bq) * g
    k = (x @ Wk + bk) * g
    v = (x @ Wv + bv) * g

    def split(t):
        return t.reshape(B, S, H, DH).transpose(0, 2, 1, 3)

    q, k, v = split(q), split(k), split(v)
    sc = np.einsum('bhqd,bhkd->bhqk', q, k) / np.float32(math.sqrt(DH))
    sc = sc - sc.max(axis=-1, keepdims=True)
    e = np.exp(sc)
    attn = e / e.sum(axis=-1, keepdims=True)
    out = np.einsum('bhqk,bhkd->bhqd', attn, v)
    out = out.transpose(0, 2, 1, 3).reshape(B, S, D)
    out = out @ Wo + bo
    return (x * (1.0 - g) + out * g).astype(np.float32)


def kernel(x, gate, Wq, bq, Wk, bk, Wv, bv, Wo, bo, _profile=None):
    x = np.asarray(x, np.float32)
    gate = np.asarray(gate)
    args = dict(x=x, gate=gate, Wq=np.asarray(Wq, np.float32),
                bq=np.asarray(bq, np.float32), Wk=np.asarray(Wk, np.float32),
                bk=np.asarray(bk, np.float32), Wv=np.asarray(Wv, np.float32),
                bv=np.asarray(bv, np.float32), Wo=np.asarray(Wo, np.float32),
                bo=np.asarray(bo, np.float32))

    idxs = [np.nonzero(gate[b])[0] for b in range(B)]
    n_act = [len(i) for i in idxs]
    # the compaction trick needs zero q/k/v biases and at least one active
    # and one inactive token per batch; otherwise fall back to exact numpy
    if (any(np.abs(args[k]).max() > 0 for k in ("bq", "bk", "bv"))
            or min(n_act) == 0 or max(n_act) == S):
        return _reference_fallback(**args)

    SA = ((max(n_act) + 127) // 128) * 128
    npdt = _NPDT[MM_DTYPE]
    xdt = ml_dtypes.float8_e4m3 if PROJ_FP8 else npdt
    ws = WSCALE if PROJ_FP8 else 1.0
    perm = _score_perm() if SCORE_FP8 else np.arange(256)

    in_maps = []
    for b in range(B):
        xa = np.zeros((SA, D), np.float32)
        xa[:n_act[b]] = x[b, idxs[b]]
        xT = np.ascontiguousarray(xa.T).astype(xdt)
        for g in range(GROUPS):
            cs = slice(g * DC, (g + 1) * DC)
            in_maps.append({
                "xT": xT,
                "wq": np.ascontiguousarray(
                    args["Wq"][:, cs][:, perm] * ws).astype(xdt),
                "wk": np.ascontiguousarray(
                    args["Wk"][:, cs][:, perm] * ws).astype(xdt),
                "wv": np.ascontiguousarray(
                    args["Wv"][:, cs] * ws).astype(xdt),
                "wo": np.ascontiguousarray(
                    args["Wo"][cs, :] * (WSCALE if Y_FP8 else 1.0)).astype(
                    ml_dtypes.float8_e4m3 if Y_FP8 else npdt),
            })

    nc = _get_nc(SA)
    kw = dict(_profile) if _profile else {}
    kw.pop("result", None)
    res = run_bass_kernel_spmd(nc, in_maps, core_ids=list(range(NCORES)), **kw)
    if _profile is not None:
        _profile["result"] = res

    out = x.copy()
    for b in range(B):
        Y = np.zeros((SA, D), np.float32)
        for g in range(GROUPS):
            Y += res.results[b * GROUPS + g]["y"]
        out[b, idxs[b]] = Y[:n_act[b]] + args["bo"]
    return out
